# revision 1
# baseline (speedup 1.0000x reference)
import sys

sys.path.insert(0, "/opt/trn_rl_repo")

import numpy as np

import concourse.bass as bass
import concourse.bacc as bacc
import concourse.mybir as mybir
from concourse.tile import TileContext
from concourse.bass_utils import run_bass_kernel_spmd

P = 128          # partitions
BT = 512         # batch-tile (free dim) per matmul
G = 4            # batch groups packed into 128 partitions for the GRU
NCORES = 8
B, S, H, A = 131072, 256, 512, 32
BC = B // NCORES           # 16384 rows per core
MACRO = G * BT             # 2048 rows per GRU macro-tile
NM = BC // MACRO           # 8 macro-tiles per core

FP32 = mybir.dt.float32
AF = mybir.ActivationFunctionType
OP = mybir.AluOpType

_CACHE = {}


def _build(nsteps: int) -> bass.Bass:
    nc = bacc.Bacc("TRN2", target_bir_lowering=False, debug=False,
                   num_devices=NCORES)

    xT = nc.dram_tensor("xT", [S, BC], FP32, kind="ExternalInput")
    w1t = nc.dram_tensor("w1t", [S, H], FP32, kind="ExternalInput")
    w2t = nc.dram_tensor("w2t", [H, H], FP32, kind="ExternalInput")
    wmt = nc.dram_tensor("wmt", [H, A], FP32, kind="ExternalInput")
    b1d = nc.dram_tensor("b1d", [P, 4], FP32, kind="ExternalInput")
    b2d = nc.dram_tensor("b2d", [P, 4], FP32, kind="ExternalInput")
    bmd = nc.dram_tensor("bmd", [A, 1], FP32, kind="ExternalInput")
    lri = nc.dram_tensor("lri", [2 * G, P], FP32, kind="ExternalInput")
    lui = nc.dram_tensor("lui", [2 * G, P], FP32, kind="ExternalInput")
    lni = nc.dram_tensor("lni", [2 * G, P], FP32, kind="ExternalInput")
    lrh = nc.dram_tensor("lrh", [P, P], FP32, kind="ExternalInput")
    luh = nc.dram_tensor("luh", [P, P], FP32, kind="ExternalInput")
    lnh = nc.dram_tensor("lnh", [P, P], FP32, kind="ExternalInput")
    lwd = nc.dram_tensor("lwd", [P, 2 * G], FP32, kind="ExternalInput")
    brd = nc.dram_tensor("brd", [P, 1], FP32, kind="ExternalInput")
    bud = nc.dram_tensor("bud", [P, 1], FP32, kind="ExternalInput")
    bnhd = nc.dram_tensor("bnhd", [P, 1], FP32, kind="ExternalInput")
    bnid = nc.dram_tensor("bnid", [P, 1], FP32, kind="ExternalInput")
    bwd = nc.dram_tensor("bwd", [2 * G, 1], FP32, kind="ExternalInput")
    # packed device layout: [macro, 8t+2g+c, j]; host unscrambles to [B, 2T]
    outT = nc.dram_tensor("outT", [NM, 2 * G * nsteps, BT], FP32,
                          kind="ExternalOutput")

    xv = xT.rearrange("(kb p) b -> p kb b", p=P)              # [128, 2, BC]
    ov = outT

    with TileContext(nc) as tc:
        with (
            tc.tile_pool(name="const", bufs=1) as const,
            tc.tile_pool(name="xp", bufs=3) as xp,
            tc.tile_pool(name="actp", bufs=2) as actp,
            tc.tile_pool(name="grup", bufs=2) as grup,
            tc.tile_pool(name="outp", bufs=2) as outp,
            tc.tile_pool(name="mps", bufs=2, space="PSUM") as mps,
            tc.tile_pool(name="zps", bufs=1, space="PSUM") as zps,
            tc.tile_pool(name="gps", bufs=1, space="PSUM") as gps,
        ):
            w1s = const.tile([P, 2, H], FP32)
            nc.sync.dma_start(w1s[:], w1t.rearrange("(kb p) f -> p kb f", p=P))
            w2s = const.tile([P, 4, H], FP32)
            nc.sync.dma_start(w2s[:], w2t.rearrange("(kb p) f -> p kb f", p=P))
            wms = const.tile([P, 4, A], FP32)
            nc.sync.dma_start(wms[:], wmt.rearrange("(kb p) f -> p kb f", p=P))
            b1s = const.tile([P, 4], FP32)
            nc.sync.dma_start(b1s[:], b1d[:])
            b2s = const.tile([P, 4], FP32)
            nc.sync.dma_start(b2s[:], b2d[:])
            bms = const.tile([A, 1], FP32)
            nc.sync.dma_start(bms[:], bmd[:])
            lris = const.tile([2 * G, P], FP32)
            nc.sync.dma_start(lris[:], lri[:])
            luis = const.tile([2 * G, P], FP32)
            nc.sync.dma_start(luis[:], lui[:])
            lnis = const.tile([2 * G, P], FP32)
            nc.sync.dma_start(lnis[:], lni[:])
            lrhs = const.tile([P, P], FP32)
            nc.sync.dma_start(lrhs[:], lrh[:])
            luhs = const.tile([P, P], FP32)
            nc.sync.dma_start(luhs[:], luh[:])
            lnhs = const.tile([P, P], FP32)
            nc.sync.dma_start(lnhs[:], lnh[:])
            lws = const.tile([P, 2 * G], FP32)
            nc.sync.dma_start(lws[:], lwd[:])
            brs = const.tile([P, 1], FP32)
            nc.sync.dma_start(brs[:], brd[:])
            bus = const.tile([P, 1], FP32)
            nc.sync.dma_start(bus[:], bud[:])
            bnhs = const.tile([P, 1], FP32)
            nc.sync.dma_start(bnhs[:], bnhd[:])
            bnis = const.tile([P, 1], FP32)
            nc.sync.dma_start(bnis[:], bnid[:])
            bws = const.tile([2 * G, 1], FP32)
            nc.sync.dma_start(bws[:], bwd[:])

            for m in range(NM):
                Z = grup.tile([P, BT], FP32, tag="Z")
                for g in range(G):
                    c0 = m * MACRO + g * BT
                    X = xp.tile([P, 2, BT], FP32, tag="X")
                    nc.sync.dma_start(X[:], xv[:, :, c0:c0 + BT])
                    H1 = actp.tile([P, 4, BT], FP32, tag="H1")
                    for f in range(4):
                        ps = mps.tile([P, BT], FP32, tag="ps")
                        nc.tensor.matmul(ps[:], w1s[:, 0, f * P:(f + 1) * P],
                                         X[:, 0, :], start=True, stop=False)
                        nc.tensor.matmul(ps[:], w1s[:, 1, f * P:(f + 1) * P],
                                         X[:, 1, :], start=False, stop=True)
                        nc.scalar.activation(H1[:, f, :], ps[:], AF.Relu,
                                             bias=b1s[:, f:f + 1])
                    H2 = actp.tile([P, 4, BT], FP32, tag="H2")
                    for f in range(4):
                        ps = mps.tile([P, BT], FP32, tag="ps")
                        for k in range(4):
                            nc.tensor.matmul(ps[:], w2s[:, k, f * P:(f + 1) * P],
                                             H1[:, k, :], start=(k == 0),
                                             stop=(k == 3))
                        nc.scalar.activation(H2[:, f, :], ps[:], AF.Relu,
                                             bias=b2s[:, f:f + 1])
                    ps3 = zps.tile([A, BT], FP32, tag="ps3")
                    for k in range(4):
                        nc.tensor.matmul(ps3[:], wms[:, k, :], H2[:, k, :],
                                         start=(k == 0), stop=(k == 3))
                    nc.scalar.activation(Z[g * A:(g + 1) * A, :], ps3[:],
                                         AF.Identity, bias=bms[:, :1])

                WP = grup.tile([2 * G, BT], FP32, tag="WP")
                nc.any.memset(WP[:], 0.0)
                wp_cur = WP[:]
                for t in range(nsteps):
                    psR = gps.tile([P, BT], FP32, tag="psR")
                    psU = gps.tile([P, BT], FP32, tag="psU")
                    psNI = gps.tile([P, BT], FP32, tag="psNI")
                    psNH = gps.tile([P, BT], FP32, tag="psNH")
                    nc.tensor.matmul(psR[:], lris[:], wp_cur, start=True, stop=False)
                    nc.tensor.matmul(psR[:], lrhs[:], Z[:], start=False, stop=True)
                    nc.tensor.matmul(psU[:], luis[:], wp_cur, start=True, stop=False)
                    nc.tensor.matmul(psU[:], luhs[:], Z[:], start=False, stop=True)
                    nc.tensor.matmul(psNI[:], lnis[:], wp_cur, start=True, stop=True)
                    nc.tensor.matmul(psNH[:], lnhs[:], Z[:], start=True, stop=True)
                    R = grup.tile([P, BT], FP32, tag="R")
                    U = grup.tile([P, BT], FP32, tag="U")
                    HN = grup.tile([P, BT], FP32, tag="HN")
                    NT = grup.tile([P, BT], FP32, tag="NT")
                    nc.scalar.activation(R[:], psR[:], AF.Sigmoid, bias=brs[:, :1])
                    nc.scalar.activation(U[:], psU[:], AF.Sigmoid, bias=bus[:, :1])
                    nc.scalar.activation(HN[:], psNH[:], AF.Identity, bias=bnhs[:, :1])
                    nc.vector.tensor_tensor(R[:], R[:], HN[:], OP.mult)
                    nc.vector.tensor_tensor(R[:], R[:], psNI[:], OP.add)
                    nc.scalar.activation(NT[:], R[:], AF.Tanh, bias=bnis[:, :1])
                    nc.vector.tensor_tensor(Z[:], Z[:], NT[:], OP.subtract)
                    nc.vector.tensor_tensor(Z[:], U[:], Z[:], OP.mult)
                    nc.vector.tensor_tensor(Z[:], Z[:], NT[:], OP.add)
                    psW = gps.tile([2 * G, BT], FP32, tag="psW")
                    nc.tensor.matmul(psW[:], lws[:], Z[:], start=True, stop=True)
                    wp_next = outp.tile([2 * G, BT], FP32, tag="WPN")
                    nc.scalar.activation(wp_next[:], psW[:], AF.Identity,
                                         bias=bws[:, :1])
                    nc.vector.tensor_tensor(wp_next[:], wp_next[:], wp_cur, OP.add)
                    nc.sync.dma_start(ov[m, 2 * G * t:2 * G * (t + 1), :],
                                      wp_next[:])
                    wp_cur = wp_next[:]
    nc.compile()
    return nc


LAST_RESULT = None


def kernel(**inputs) -> np.ndarray:
    global LAST_RESULT
    x = np.ascontiguousarray(np.asarray(inputs["x"], dtype=np.float32))
    W1 = np.asarray(inputs["W1"], np.float32)
    b1 = np.asarray(inputs["b1"], np.float32)
    W2 = np.asarray(inputs["W2"], np.float32)
    b2 = np.asarray(inputs["b2"], np.float32)
    Wm = np.asarray(inputs["Wm"], np.float32)
    bm = np.asarray(inputs["bm"], np.float32)
    w_ih = np.asarray(inputs["w_ih"], np.float32)
    w_hh = np.asarray(inputs["w_hh"], np.float32)
    b_ih = np.asarray(inputs["b_ih"], np.float32)
    b_hh = np.asarray(inputs["b_hh"], np.float32)
    Ww = np.asarray(inputs["Ww"], np.float32)
    bw = np.asarray(inputs["bw"], np.float32)
    T = int(inputs["pred_length"])

    I4 = np.eye(G, dtype=np.float32)
    common = {
        "w1t": np.ascontiguousarray(W1.T),
        "w2t": np.ascontiguousarray(W2.T),
        "wmt": np.ascontiguousarray(Wm.T),
        "b1d": np.ascontiguousarray(b1.reshape(4, P).T),
        "b2d": np.ascontiguousarray(b2.reshape(4, P).T),
        "bmd": bm.reshape(A, 1).copy(),
        "lri": np.ascontiguousarray(np.kron(I4, w_ih[0:A].T)),
        "lui": np.ascontiguousarray(np.kron(I4, w_ih[A:2 * A].T)),
        "lni": np.ascontiguousarray(np.kron(I4, w_ih[2 * A:3 * A].T)),
        "lrh": np.ascontiguousarray(np.kron(I4, w_hh[0:A].T)),
        "luh": np.ascontiguousarray(np.kron(I4, w_hh[A:2 * A].T)),
        "lnh": np.ascontiguousarray(np.kron(I4, w_hh[2 * A:3 * A].T)),
        "lwd": np.ascontiguousarray(np.kron(I4, Ww.T)),
        "brd": np.tile(b_ih[0:A] + b_hh[0:A], G).reshape(P, 1).copy(),
        "bud": np.tile(b_ih[A:2 * A] + b_hh[A:2 * A], G).reshape(P, 1).copy(),
        "bnhd": np.tile(b_hh[2 * A:3 * A], G).reshape(P, 1).copy(),
        "bnid": np.tile(b_ih[2 * A:3 * A], G).reshape(P, 1).copy(),
        "bwd": np.tile(bw, G).reshape(2 * G, 1).copy(),
    }
    global _last_common
    _last_common = common
    xT = np.ascontiguousarray(x.T)          # [S, B]
    in_maps = []
    for i in range(NCORES):
        m = dict(common)
        m["xT"] = np.ascontiguousarray(xT[:, i * BC:(i + 1) * BC])
        in_maps.append(m)

    if T not in _CACHE:
        _CACHE[T] = _build(T)
    nc = _CACHE[T]
    res = run_bass_kernel_spmd(nc, in_maps, core_ids=list(range(NCORES)))
    LAST_RESULT = res
    parts = []
    for i in range(NCORES):
        o = np.asarray(res.results[i]["outT"])       # [NM, 2*G*T, BT]
        o = o.reshape(NM, T, G, 2, BT).transpose(0, 2, 4, 1, 3)
        parts.append(o.reshape(BC, 2 * T))
    return np.ascontiguousarray(np.concatenate(parts, axis=0))



# revision 7
# speedup vs baseline: 2.5611x; 2.5611x over previous
import sys

sys.path.insert(0, "/opt/trn_rl_repo")

import numpy as np
import ml_dtypes

import concourse.bass as bass
import concourse.bacc as bacc
import concourse.mybir as mybir
from concourse.tile import TileContext
from concourse.bass_utils import run_bass_kernel_spmd

P = 128          # partitions
BT = 512         # batch-tile (free dim)
G = 4            # batch groups packed into 128 partitions for the GRU
NCORES = 8
B, S, H, A = 131072, 256, 512, 32
BC = B // NCORES           # 16384 rows per core
MACRO = G * BT             # 2048 rows per GRU macro-tile
NM = BC // MACRO           # 8 macro-tiles per core

FP32 = mybir.dt.float32
BF16 = mybir.dt.bfloat16
AF = mybir.ActivationFunctionType
OP = mybir.AluOpType
BF = ml_dtypes.bfloat16

_CACHE = {}


def _build(nsteps: int) -> bass.Bass:
    nc = bacc.Bacc("TRN2", target_bir_lowering=False, debug=False,
                   num_devices=NCORES)

    xT = nc.dram_tensor("xT", [S, BC], BF16, kind="ExternalInput")
    w1t = nc.dram_tensor("w1t", [S, H], BF16, kind="ExternalInput")
    w2t = nc.dram_tensor("w2t", [H, H], BF16, kind="ExternalInput")
    wmt = nc.dram_tensor("wmt", [H, A], BF16, kind="ExternalInput")
    b1d = nc.dram_tensor("b1d", [P, 4], FP32, kind="ExternalInput")
    b2d = nc.dram_tensor("b2d", [P, 4], FP32, kind="ExternalInput")
    bmd = nc.dram_tensor("bmd", [P, 1], FP32, kind="ExternalInput")
    # i-side gate weights with bias folded in via a constant-1 ninth row
    lrid = nc.dram_tensor("lrid", [9, P], BF16, kind="ExternalInput")
    luid = nc.dram_tensor("luid", [9, P], BF16, kind="ExternalInput")
    lnid = nc.dram_tensor("lnid", [9, P], BF16, kind="ExternalInput")
    lrhd = nc.dram_tensor("lrhd", [P, P], BF16, kind="ExternalInput")
    luhd = nc.dram_tensor("luhd", [P, P], BF16, kind="ExternalInput")
    lnhd = nc.dram_tensor("lnhd", [P, P], BF16, kind="ExternalInput")
    lwd = nc.dram_tensor("lwd", [P, 2 * G], BF16, kind="ExternalInput")
    bnhd = nc.dram_tensor("bnhd", [P, 1], FP32, kind="ExternalInput")
    bwd = nc.dram_tensor("bwd", [2 * G, 1], FP32, kind="ExternalInput")
    wpinit = nc.dram_tensor("wpinit", [9, BT], BF16, kind="ExternalInput")
    outd = nc.dram_tensor("outd", [nsteps, NM, 2 * G, BT], BF16,
                          kind="ExternalOutput")

    xv = xT.rearrange("(kb p) b -> p kb b", p=P)              # [128, 2, BC]
    NIT = NM * nsteps

    with TileContext(nc) as tc:
        with (
            tc.tile_pool(name="const", bufs=1) as const,
            tc.tile_pool(name="state", bufs=1) as state,
        ):
            w1s = const.tile([P, 2, H], BF16)
            nc.sync.dma_start(w1s[:], w1t.rearrange("(kb p) f -> p kb f", p=P))
            w2s = const.tile([P, 4, H], BF16)
            nc.sync.dma_start(w2s[:], w2t.rearrange("(kb p) f -> p kb f", p=P))
            wms = const.tile([P, 4, A], BF16)
            nc.sync.dma_start(wms[:], wmt.rearrange("(kb p) f -> p kb f", p=P))
            b1s = const.tile([P, 4], FP32)
            nc.sync.dma_start(b1s[:], b1d[:])
            b2s = const.tile([P, 4], FP32)
            nc.sync.dma_start(b2s[:], b2d[:])
            bms = const.tile([P, 1], FP32)
            nc.sync.dma_start(bms[:], bmd[:])
            lris = const.tile([9, P], BF16)
            nc.sync.dma_start(lris[:], lrid[:])
            luis = const.tile([9, P], BF16)
            nc.sync.dma_start(luis[:], luid[:])
            lnis = const.tile([9, P], BF16)
            nc.sync.dma_start(lnis[:], lnid[:])
            lrhs = const.tile([P, P], BF16)
            nc.sync.dma_start(lrhs[:], lrhd[:])
            luhs = const.tile([P, P], BF16)
            nc.sync.dma_start(luhs[:], luhd[:])
            lnhs = const.tile([P, P], BF16)
            nc.sync.dma_start(lnhs[:], lnhd[:])
            lws = const.tile([P, 2 * G], BF16)
            nc.sync.dma_start(lws[:], lwd[:])
            bnhs = const.tile([P, 1], FP32)
            nc.sync.dma_start(bnhs[:], bnhd[:])
            bws = const.tile([2 * G, 1], FP32)
            nc.sync.dma_start(bws[:], bwd[:])

            Z = []
            WPS = []
            for m in range(NM):
                zt = state.tile([P, BT], BF16, tag=f"Z{m}", name=f"Z{m}")
                Z.append(zt)
                wt = state.tile([9, BT], BF16, tag=f"WP{m}", name=f"WP{m}")
                WPS.append(wt)
                nc.sync.dma_start(wt[:], wpinit[:])

            # ---------------- phase 1: MLP encoder -> Z[m] ----------------
            with (
                tc.tile_pool(name="xp", bufs=3) as xp,
                tc.tile_pool(name="hp", bufs=2) as hp,
                tc.tile_pool(name="mps", bufs=2, space="PSUM") as mps,
                tc.tile_pool(name="zps", bufs=2, space="PSUM") as zps,
            ):
                for m in range(NM):
                    psZ = zps.tile([P, BT], FP32, tag="psZ")
                    for g in range(G):
                        c0 = m * MACRO + g * BT
                        X = xp.tile([P, 2, BT], BF16, tag="X")
                        nc.sync.dma_start(X[:], xv[:, :, c0:c0 + BT])
                        H1 = hp.tile([P, 4, BT], BF16, tag="H1")
                        for f in range(4):
                            ps = mps.tile([P, BT], FP32, tag="mm")
                            nc.tensor.matmul(ps[:], w1s[:, 0, f * P:(f + 1) * P],
                                             X[:, 0, :], start=True, stop=False)
                            nc.tensor.matmul(ps[:], w1s[:, 1, f * P:(f + 1) * P],
                                             X[:, 1, :], start=False, stop=True)
                            if f % 2 == 0:
                                nc.scalar.activation(H1[:, f, :], ps[:], AF.Relu,
                                                     bias=b1s[:, f:f + 1])
                            else:
                                nc.vector.tensor_scalar(H1[:, f, :], ps[:],
                                                        b1s[:, f:f + 1], 0.0,
                                                        OP.add, OP.max)
                        H2 = hp.tile([P, 4, BT], BF16, tag="H2")
                        for f in range(4):
                            ps = mps.tile([P, BT], FP32, tag="mm")
                            for k in range(4):
                                nc.tensor.matmul(ps[:], w2s[:, k, f * P:(f + 1) * P],
                                                 H1[:, k, :], start=(k == 0),
                                                 stop=(k == 3))
                            if f % 2 == 0:
                                nc.scalar.activation(H2[:, f, :], ps[:], AF.Relu,
                                                     bias=b2s[:, f:f + 1])
                            else:
                                nc.vector.tensor_scalar(H2[:, f, :], ps[:],
                                                        b2s[:, f:f + 1], 0.0,
                                                        OP.add, OP.max)
                        for k in range(4):
                            nc.tensor.matmul(psZ[g * A:(g + 1) * A, :],
                                             wms[:, k, :], H2[:, k, :],
                                             start=(k == 0), stop=(k == 3),
                                             tile_position=(0, g * A))
                    nc.scalar.activation(Z[m][:], psZ[:], AF.Identity,
                                         bias=bms[:, 0:1])

            # ---------------- phase 2: GRU, software-pipelined ----------------
            # iter i = t*NM + m; stages skewed: gates/RU/P1/P2 at slot i,
            # NT + z-update at slot i+2, psW/wp-update/DMA at slot i+3.
            with (
                tc.tile_pool(name="rup", bufs=3) as rup,
                tc.tile_pool(name="ntp", bufs=2) as ntp,
                tc.tile_pool(name="p1p", bufs=2) as p1p,
                tc.tile_pool(name="p2p", bufs=3) as p2p,
                tc.tile_pool(name="dp", bufs=2) as dp,
                tc.tile_pool(name="gps", bufs=1, space="PSUM") as gps,
                tc.tile_pool(name="nhp", bufs=2, space="PSUM") as nhp,
                tc.tile_pool(name="nip", bufs=2, space="PSUM") as nip,
                tc.tile_pool(name="wpp", bufs=2, space="PSUM") as wpp,
            ):
                RUs = {}
                NTs = {}
                P1s = {}
                P2s = {}
                PWs = {}
                Ds = {}
                Es = {}
                psRUs = {}
                psNHs = {}
                psNIs = {}
                for i in range(NIT + 4):
                    j2 = i - 2
                    j3 = i - 3
                    # PE: gate matmuls for iter i
                    if i < NIT:
                        m = i % NM
                        psRU = gps.tile([P, 2 * BT], FP32, tag="psRU")
                        nc.tensor.matmul(psRU[:, 0:BT], lris[:], WPS[m][:],
                                         start=True, stop=False)
                        nc.tensor.matmul(psRU[:, 0:BT], lrhs[:], Z[m][:],
                                         start=False, stop=True)
                        nc.tensor.matmul(psRU[:, BT:2 * BT], luis[:], WPS[m][:],
                                         start=True, stop=False)
                        nc.tensor.matmul(psRU[:, BT:2 * BT], luhs[:], Z[m][:],
                                         start=False, stop=True)
                        psNH = nhp.tile([P, BT], FP32, tag="psNH")
                        nc.tensor.matmul(psNH[:], lnhs[:], Z[m][:],
                                         start=True, stop=True)
                        psNI = nip.tile([P, BT], FP32, tag="psNI")
                        nc.tensor.matmul(psNI[:], lnis[:], WPS[m][:],
                                         start=True, stop=True)
                        psRUs[i] = psRU
                        psNHs[i] = psNH
                        psNIs[i] = psNI
                    # PE: wp matmul for iter i-3 (Z[m3] holds updated z)
                    if 0 <= j3 < NIT:
                        m3 = j3 % NM
                        psW = wpp.tile([2 * G, BT], FP32, tag="psW")
                        nc.tensor.matmul(psW[:], lws[:], Z[m3][:],
                                         start=True, stop=True)
                        PWs[j3] = psW
                    # Act: tanh for iter i-2, then fused sigmoid(R|U) for iter i
                    if 0 <= j2 < NIT:
                        NT = ntp.tile([P, BT], BF16, tag="NT")
                        nc.scalar.activation(NT[:], P2s[j2][:], AF.Tanh)
                        NTs[j2] = NT
                    if i < NIT:
                        RU = rup.tile([P, 2 * BT], BF16, tag="RU")
                        nc.scalar.activation(RU[:], psRUs[i][:], AF.Sigmoid)
                        RUs[i] = RU
                    # Pool (SBUF only): d = z - nt, e = u * d for iter i-2
                    if 0 <= j2 < NIT:
                        m2 = j2 % NM
                        D = dp.tile([P, BT], BF16, tag="D")
                        nc.gpsimd.tensor_tensor(D[:], Z[m2][:], NTs[j2][:],
                                                OP.subtract)
                        Ds[j2] = D
                        E = dp.tile([P, BT], BF16, tag="E")
                        nc.gpsimd.tensor_tensor(E[:], RUs[j2][:, BT:2 * BT],
                                                D[:], OP.mult)
                        Es[j2] = E
                    # DVE: P1(i); z(i-2) = nt + e; P2(i); wp(i-3)
                    if i < NIT:
                        P1 = p1p.tile([P, BT], FP32, tag="P1")
                        nc.vector.scalar_tensor_tensor(P1[:], psNHs[i][:],
                                                       bnhs[:, 0:1],
                                                       RUs[i][:, 0:BT],
                                                       OP.add, OP.mult)
                        P1s[i] = P1
                    if 0 <= j2 < NIT:
                        m2 = j2 % NM
                        nc.vector.scalar_tensor_tensor(Z[m2][:], NTs[j2][:],
                                                       0.0, Es[j2][:],
                                                       OP.add, OP.add)
                    if i < NIT:
                        P2 = p2p.tile([P, BT], FP32, tag="P2")
                        nc.vector.scalar_tensor_tensor(P2[:], psNIs[i][:], 0.0,
                                                       P1s[i][:], OP.add, OP.add)
                        P2s[i] = P2
                    # DVE: wp += psW + bw for iter i-3, then stream out
                    if 0 <= j3 < NIT:
                        t3, m3 = divmod(j3, NM)
                        nc.vector.scalar_tensor_tensor(WPS[m3][0:8, :],
                                                       PWs[j3][:], bws[:, 0:1],
                                                       WPS[m3][0:8, :],
                                                       OP.add, OP.add)
                        nc.sync.dma_start(outd[t3, m3, :, :], WPS[m3][0:8, :])
    nc.compile()
    return nc


LAST_RESULT = None


def kernel(**inputs) -> np.ndarray:
    global LAST_RESULT
    x = np.asarray(inputs["x"], np.float32)
    W1 = np.asarray(inputs["W1"], np.float32)
    b1 = np.asarray(inputs["b1"], np.float32)
    W2 = np.asarray(inputs["W2"], np.float32)
    b2 = np.asarray(inputs["b2"], np.float32)
    Wm = np.asarray(inputs["Wm"], np.float32)
    bm = np.asarray(inputs["bm"], np.float32)
    w_ih = np.asarray(inputs["w_ih"], np.float32)
    w_hh = np.asarray(inputs["w_hh"], np.float32)
    b_ih = np.asarray(inputs["b_ih"], np.float32)
    b_hh = np.asarray(inputs["b_hh"], np.float32)
    Ww = np.asarray(inputs["Ww"], np.float32)
    bw = np.asarray(inputs["bw"], np.float32)
    T = int(inputs["pred_length"])

    I4 = np.eye(G, dtype=np.float32)

    def pack9(wg, bias):
        # [9, 128]: rows 0-7 block-diag i-weights, row 8 the folded bias
        mret = np.zeros((9, P), np.float32)
        mret[0:8, :] = np.kron(I4, wg.T)
        mret[8, :] = np.tile(bias, G)
        return mret.astype(BF)

    common = {
        "w1t": W1.T.astype(BF),
        "w2t": W2.T.astype(BF),
        "wmt": Wm.T.astype(BF),
        "b1d": np.ascontiguousarray(b1.reshape(4, P).T),
        "b2d": np.ascontiguousarray(b2.reshape(4, P).T),
        "bmd": np.tile(bm, G).reshape(P, 1).copy(),
        "lrid": pack9(w_ih[0:A], b_ih[0:A] + b_hh[0:A]),
        "luid": pack9(w_ih[A:2 * A], b_ih[A:2 * A] + b_hh[A:2 * A]),
        "lnid": pack9(w_ih[2 * A:3 * A], b_ih[2 * A:3 * A]),
        "lrhd": np.kron(I4, w_hh[0:A].T).astype(BF),
        "luhd": np.kron(I4, w_hh[A:2 * A].T).astype(BF),
        "lnhd": np.kron(I4, w_hh[2 * A:3 * A].T).astype(BF),
        "lwd": np.kron(I4, Ww.T).astype(BF),
        "bnhd": np.tile(b_hh[2 * A:3 * A], G).reshape(P, 1).copy(),
        "bwd": np.tile(bw, G).reshape(2 * G, 1).copy(),
        "wpinit": np.concatenate(
            [np.zeros((8, BT), np.float32), np.ones((1, BT), np.float32)]
        ).astype(BF),
    }
    xTb = x.T.astype(BF)                     # [S, B]
    in_maps = []
    for i in range(NCORES):
        m = dict(common)
        m["xT"] = np.ascontiguousarray(xTb[:, i * BC:(i + 1) * BC])
        in_maps.append(m)

    if T not in _CACHE:
        _CACHE[T] = _build(T)
    nc = _CACHE[T]
    res = run_bass_kernel_spmd(nc, in_maps, core_ids=list(range(NCORES)))
    LAST_RESULT = res
    parts = []
    for i in range(NCORES):
        o = np.asarray(res.results[i]["outd"]).astype(np.float32)
        # [T, NM, 2G, BT] -> rows m*2048 + g*512 + c, cols 2t+j
        o = o.reshape(T, NM, G, 2, BT).transpose(1, 2, 4, 0, 3)
        parts.append(o.reshape(BC, 2 * T))
    return np.ascontiguousarray(np.concatenate(parts, axis=0))


# revision 10
# speedup vs baseline: 2.6831x; 1.0477x over previous
import sys

sys.path.insert(0, "/opt/trn_rl_repo")

import numpy as np
import ml_dtypes

import concourse.bass as bass
import concourse.bacc as bacc
import concourse.mybir as mybir
from concourse.tile import TileContext
from concourse.bass_utils import run_bass_kernel_spmd

P = 128          # partitions
BT = 512         # batch-tile (free dim)
G = 4            # batch groups packed into 128 partitions for the GRU
NCORES = 8
B, S, H, A = 131072, 256, 512, 32
BC = B // NCORES           # 16384 rows per core
MACRO = G * BT             # 2048 rows per GRU macro-tile
NM = BC // MACRO           # 8 macro-tiles per core

FP32 = mybir.dt.float32
BF16 = mybir.dt.bfloat16
AF = mybir.ActivationFunctionType
OP = mybir.AluOpType
BF = ml_dtypes.bfloat16

_CACHE = {}


def _build(nsteps: int) -> bass.Bass:
    nc = bacc.Bacc("TRN2", target_bir_lowering=False, debug=False,
                   num_devices=NCORES)

    xT = nc.dram_tensor("xT", [S, BC], BF16, kind="ExternalInput")
    w1t = nc.dram_tensor("w1t", [S, H], BF16, kind="ExternalInput")
    w2t = nc.dram_tensor("w2t", [H, H], BF16, kind="ExternalInput")
    wmt = nc.dram_tensor("wmt", [H, A], BF16, kind="ExternalInput")
    b1d = nc.dram_tensor("b1d", [P, 4], FP32, kind="ExternalInput")
    b2d = nc.dram_tensor("b2d", [P, 4], FP32, kind="ExternalInput")
    bmd = nc.dram_tensor("bmd", [P, 1], FP32, kind="ExternalInput")
    # i-side gate weights with bias folded in via a constant-1 ninth row
    lrid = nc.dram_tensor("lrid", [9, P], BF16, kind="ExternalInput")
    luid = nc.dram_tensor("luid", [9, P], BF16, kind="ExternalInput")
    lnid = nc.dram_tensor("lnid", [9, P], BF16, kind="ExternalInput")
    lrhd = nc.dram_tensor("lrhd", [P, P], BF16, kind="ExternalInput")
    luhd = nc.dram_tensor("luhd", [P, P], BF16, kind="ExternalInput")
    lnhd = nc.dram_tensor("lnhd", [P, P], BF16, kind="ExternalInput")
    lwd = nc.dram_tensor("lwd", [P, 2 * G], BF16, kind="ExternalInput")
    bnhd = nc.dram_tensor("bnhd", [P, 1], FP32, kind="ExternalInput")
    bwd = nc.dram_tensor("bwd", [2 * G, 1], FP32, kind="ExternalInput")
    wpinit = nc.dram_tensor("wpinit", [9, BT], BF16, kind="ExternalInput")
    outd = nc.dram_tensor("outd", [nsteps, NM, 2 * G, BT], BF16,
                          kind="ExternalOutput")

    xv = xT.rearrange("(kb p) b -> p kb b", p=P)              # [128, 2, BC]

    with TileContext(nc) as tc:
        with (
            tc.tile_pool(name="const", bufs=1) as const,
            tc.tile_pool(name="state", bufs=1) as state,
            tc.tile_pool(name="xp", bufs=3) as xp,
            tc.tile_pool(name="hp", bufs=2) as hp,
            tc.tile_pool(name="rup", bufs=3) as rup,
            tc.tile_pool(name="ntp", bufs=2) as ntp,
            tc.tile_pool(name="p1p", bufs=2) as p1p,
            tc.tile_pool(name="p2p", bufs=3) as p2p,
            tc.tile_pool(name="dp", bufs=2) as dp,
            tc.tile_pool(name="mps", bufs=2, space="PSUM") as mps,
            tc.tile_pool(name="zps", bufs=1, space="PSUM") as zps,
            tc.tile_pool(name="gps", bufs=1, space="PSUM") as gps,
            tc.tile_pool(name="nhp", bufs=1, space="PSUM") as nhp,
            tc.tile_pool(name="nip", bufs=1, space="PSUM") as nip,
            tc.tile_pool(name="wpp", bufs=1, space="PSUM") as wpp,
        ):
            w1s = const.tile([P, 2, H], BF16)
            nc.sync.dma_start(w1s[:], w1t.rearrange("(kb p) f -> p kb f", p=P))
            w2s = const.tile([P, 4, H], BF16)
            nc.sync.dma_start(w2s[:], w2t.rearrange("(kb p) f -> p kb f", p=P))
            wms = const.tile([P, 4, A], BF16)
            nc.sync.dma_start(wms[:], wmt.rearrange("(kb p) f -> p kb f", p=P))
            b1s = const.tile([P, 4], FP32)
            nc.sync.dma_start(b1s[:], b1d[:])
            b2s = const.tile([P, 4], FP32)
            nc.sync.dma_start(b2s[:], b2d[:])
            bms = const.tile([P, 1], FP32)
            nc.sync.dma_start(bms[:], bmd[:])
            lris = const.tile([9, P], BF16)
            nc.sync.dma_start(lris[:], lrid[:])
            luis = const.tile([9, P], BF16)
            nc.sync.dma_start(luis[:], luid[:])
            lnis = const.tile([9, P], BF16)
            nc.sync.dma_start(lnis[:], lnid[:])
            lrhs = const.tile([P, P], BF16)
            nc.sync.dma_start(lrhs[:], lrhd[:])
            luhs = const.tile([P, P], BF16)
            nc.sync.dma_start(luhs[:], luhd[:])
            lnhs = const.tile([P, P], BF16)
            nc.sync.dma_start(lnhs[:], lnhd[:])
            lws = const.tile([P, 2 * G], BF16)
            nc.sync.dma_start(lws[:], lwd[:])
            bnhs = const.tile([P, 1], FP32)
            nc.sync.dma_start(bnhs[:], bnhd[:])
            bws = const.tile([2 * G, 1], FP32)
            nc.sync.dma_start(bws[:], bwd[:])

            Z = []
            WPS = []
            for m in range(NM):
                zt = state.tile([P, BT], BF16, tag=f"Z{m}", name=f"Z{m}")
                Z.append(zt)
                wt = state.tile([9, BT], BF16, tag=f"WP{m}", name=f"WP{m}")
                WPS.append(wt)
                nc.sync.dma_start(wt[:], wpinit[:])

            # ------------- emission helpers -------------
            psZs = [None] * NM

            def emit_mlp_group(m, g):
                c0 = m * MACRO + g * BT
                X = xp.tile([P, 2, BT], BF16, tag="X", name="X")
                nc.sync.dma_start(X[:], xv[:, :, c0:c0 + BT])
                H1 = hp.tile([P, 4, BT], BF16, tag="H1", name="H1")
                for f in range(4):
                    ps = mps.tile([P, BT], FP32, tag="mm", name="ps")
                    nc.tensor.matmul(ps[:], w1s[:, 0, f * P:(f + 1) * P],
                                     X[:, 0, :], start=True, stop=False)
                    nc.tensor.matmul(ps[:], w1s[:, 1, f * P:(f + 1) * P],
                                     X[:, 1, :], start=False, stop=True)
                    if f % 2 == 0:
                        nc.scalar.activation(H1[:, f, :], ps[:], AF.Relu,
                                             bias=b1s[:, f:f + 1])
                    else:
                        nc.vector.tensor_scalar(H1[:, f, :], ps[:],
                                                b1s[:, f:f + 1], 0.0,
                                                OP.add, OP.max)
                H2 = hp.tile([P, 4, BT], BF16, tag="H2", name="H2")
                for f in range(4):
                    ps = mps.tile([P, BT], FP32, tag="mm", name="ps")
                    for k in range(4):
                        nc.tensor.matmul(ps[:], w2s[:, k, f * P:(f + 1) * P],
                                         H1[:, k, :], start=(k == 0),
                                         stop=(k == 3))
                    if f % 2 == 0:
                        nc.scalar.activation(H2[:, f, :], ps[:], AF.Relu,
                                             bias=b2s[:, f:f + 1])
                    else:
                        nc.vector.tensor_scalar(H2[:, f, :], ps[:],
                                                b2s[:, f:f + 1], 0.0,
                                                OP.add, OP.max)
                if g == 0:
                    psZs[m] = zps.tile([P, BT], FP32, tag="psZ", name="psZ")
                for k in range(4):
                    nc.tensor.matmul(psZs[m][g * A:(g + 1) * A, :],
                                     wms[:, k, :], H2[:, k, :],
                                     start=(k == 0), stop=(k == 3),
                                     tile_position=(0, g * A))
                if g == G - 1:
                    nc.scalar.activation(Z[m][:], psZs[m][:], AF.Identity,
                                         bias=bms[:, 0:1])

            # GRU iteration state, staged across issue-slots:
            #  A(q): gate matmuls + RU sigmoid + P1 + P2
            #  B(q): NT tanh + d/e/znew + psW matmul     (1 slot later)
            #  C(q): wp accumulate + output DMA          (2 slots later)
            class It:
                __slots__ = ("t", "m", "psRU", "psNH", "psNI", "RU", "P1",
                             "P2", "NT", "psW")

            def emit_A_gates(q):
                m = q.m
                q.psRU = gps.tile([P, 2 * BT], FP32, tag="psRU", name="psRU")
                nc.tensor.matmul(q.psRU[:, 0:BT], lris[:], WPS[m][:],
                                 start=True, stop=False)
                nc.tensor.matmul(q.psRU[:, 0:BT], lrhs[:], Z[m][:],
                                 start=False, stop=True)
                nc.tensor.matmul(q.psRU[:, BT:2 * BT], luis[:], WPS[m][:],
                                 start=True, stop=False)
                nc.tensor.matmul(q.psRU[:, BT:2 * BT], luhs[:], Z[m][:],
                                 start=False, stop=True)
                q.psNH = nhp.tile([P, BT], FP32, tag="psNH", name="psNH")
                nc.tensor.matmul(q.psNH[:], lnhs[:], Z[m][:],
                                 start=True, stop=True)
                q.psNI = nip.tile([P, BT], FP32, tag="psNI", name="psNI")
                nc.tensor.matmul(q.psNI[:], lnis[:], WPS[m][:],
                                 start=True, stop=True)

            def emit_A_rest(q):
                q.RU = rup.tile([P, 2 * BT], BF16, tag="RU", name="RU")
                nc.scalar.activation(q.RU[:], q.psRU[:], AF.Sigmoid)
                q.P1 = p1p.tile([P, BT], FP32, tag="P1", name="P1")
                nc.vector.scalar_tensor_tensor(q.P1[:], q.psNH[:],
                                               bnhs[:, 0:1], q.RU[:, 0:BT],
                                               OP.add, OP.mult)
                q.P2 = p2p.tile([P, BT], FP32, tag="P2", name="P2")
                nc.vector.scalar_tensor_tensor(q.P2[:], q.psNI[:], 0.0,
                                               q.P1[:], OP.add, OP.add)

            def emit_B(q):
                m = q.m
                q.NT = ntp.tile([P, BT], BF16, tag="NT", name="NT")
                nc.scalar.activation(q.NT[:], q.P2[:], AF.Tanh)
                D = dp.tile([P, BT], BF16, tag="D", name="D")
                nc.vector.scalar_tensor_tensor(D[:], Z[m][:], 0.0, q.NT[:],
                                               OP.add, OP.subtract)
                E = dp.tile([P, BT], BF16, tag="E", name="E")
                nc.vector.scalar_tensor_tensor(E[:], q.RU[:, BT:2 * BT], 0.0,
                                               D[:], OP.add, OP.mult)
                nc.vector.scalar_tensor_tensor(Z[m][:], q.NT[:], 0.0, E[:],
                                               OP.add, OP.add)
                q.psW = wpp.tile([2 * G, BT], FP32, tag="psW", name="psW")
                nc.tensor.matmul(q.psW[:], lws[:], Z[m][:],
                                 start=True, stop=True)

            def emit_C(q):
                m = q.m
                nc.vector.scalar_tensor_tensor(WPS[m][0:8, :], q.psW[:],
                                               bws[:, 0:1], WPS[m][0:8, :],
                                               OP.add, OP.add)
                nc.sync.dma_start(outd[q.t, m, :, :], WPS[m][0:8, :])

            # ------------- merged list schedule -------------
            next_t = [0] * NM
            last_slot = [-10] * NM
            mlp_done = [False] * NM
            b_queue = []        # (slot_of_A, It)
            c_queue = []        # (slot_of_B, It)
            slot = 0

            def gru_slot():
                nonlocal slot
                # pop due C first (frees psW buffer early in DVE order)
                if c_queue and c_queue[0][0] <= slot - 1:
                    emit_C(c_queue.pop(0)[1])
                # new A: greedy pick ready macro with most remaining steps
                pick = -1
                best = 0
                for m in range(NM):
                    if (mlp_done[m] and next_t[m] < nsteps
                            and last_slot[m] <= slot - 2):
                        rem = nsteps - next_t[m]
                        if rem > best:
                            best = rem
                            pick = m
                qa = None
                if pick >= 0:
                    qa = It()
                    qa.t = next_t[pick]
                    qa.m = pick
                    next_t[pick] += 1
                    last_slot[pick] = slot
                    emit_A_gates(qa)
                # due B (deps all from earlier slots -> engines see ready work)
                if b_queue and b_queue[0][0] <= slot - 1:
                    qb = b_queue.pop(0)[1]
                    emit_B(qb)
                    c_queue.append((slot, qb))
                if qa is not None:
                    emit_A_rest(qa)
                    b_queue.append((slot, qa))
                slot += 1

            for s in range(NM * G):
                emit_mlp_group(s // G, s % G)
                if s % G == G - 1:
                    mlp_done[s // G] = True
                for _ in range(3):
                    gru_slot()
            while (any(next_t[m] < nsteps for m in range(NM))
                   or b_queue or c_queue):
                gru_slot()
    nc.compile()
    return nc


LAST_RESULT = None


def kernel(**inputs) -> np.ndarray:
    global LAST_RESULT
    x = np.asarray(inputs["x"], np.float32)
    W1 = np.asarray(inputs["W1"], np.float32)
    b1 = np.asarray(inputs["b1"], np.float32)
    W2 = np.asarray(inputs["W2"], np.float32)
    b2 = np.asarray(inputs["b2"], np.float32)
    Wm = np.asarray(inputs["Wm"], np.float32)
    bm = np.asarray(inputs["bm"], np.float32)
    w_ih = np.asarray(inputs["w_ih"], np.float32)
    w_hh = np.asarray(inputs["w_hh"], np.float32)
    b_ih = np.asarray(inputs["b_ih"], np.float32)
    b_hh = np.asarray(inputs["b_hh"], np.float32)
    Ww = np.asarray(inputs["Ww"], np.float32)
    bw = np.asarray(inputs["bw"], np.float32)
    T = int(inputs["pred_length"])

    I4 = np.eye(G, dtype=np.float32)

    def pack9(wg, bias):
        # [9, 128]: rows 0-7 block-diag i-weights, row 8 the folded bias
        mret = np.zeros((9, P), np.float32)
        mret[0:8, :] = np.kron(I4, wg.T)
        mret[8, :] = np.tile(bias, G)
        return mret.astype(BF)

    common = {
        "w1t": W1.T.astype(BF),
        "w2t": W2.T.astype(BF),
        "wmt": Wm.T.astype(BF),
        "b1d": np.ascontiguousarray(b1.reshape(4, P).T),
        "b2d": np.ascontiguousarray(b2.reshape(4, P).T),
        "bmd": np.tile(bm, G).reshape(P, 1).copy(),
        "lrid": pack9(w_ih[0:A], b_ih[0:A] + b_hh[0:A]),
        "luid": pack9(w_ih[A:2 * A], b_ih[A:2 * A] + b_hh[A:2 * A]),
        "lnid": pack9(w_ih[2 * A:3 * A], b_ih[2 * A:3 * A]),
        "lrhd": np.kron(I4, w_hh[0:A].T).astype(BF),
        "luhd": np.kron(I4, w_hh[A:2 * A].T).astype(BF),
        "lnhd": np.kron(I4, w_hh[2 * A:3 * A].T).astype(BF),
        "lwd": np.kron(I4, Ww.T).astype(BF),
        "bnhd": np.tile(b_hh[2 * A:3 * A], G).reshape(P, 1).copy(),
        "bwd": np.tile(bw, G).reshape(2 * G, 1).copy(),
        "wpinit": np.concatenate(
            [np.zeros((8, BT), np.float32), np.ones((1, BT), np.float32)]
        ).astype(BF),
    }
    xTb = x.T.astype(BF)                     # [S, B]
    in_maps = []
    for i in range(NCORES):
        m = dict(common)
        m["xT"] = np.ascontiguousarray(xTb[:, i * BC:(i + 1) * BC])
        in_maps.append(m)

    if T not in _CACHE:
        _CACHE[T] = _build(T)
    nc = _CACHE[T]
    res = run_bass_kernel_spmd(nc, in_maps, core_ids=list(range(NCORES)))
    LAST_RESULT = res
    parts = []
    for i in range(NCORES):
        o = np.asarray(res.results[i]["outd"]).astype(np.float32)
        # [T, NM, 2G, BT] -> rows m*2048 + g*512 + c, cols 2t+j
        o = o.reshape(T, NM, G, 2, BT).transpose(1, 2, 4, 0, 3)
        parts.append(o.reshape(BC, 2 * T))
    return np.ascontiguousarray(np.concatenate(parts, axis=0))
